# revision 21
# baseline (speedup 1.0000x reference)
"""Trainium2 Bass kernel for masked attention + LayerNorm (nn_Attention_4183298146361).

Per-core (data-parallel over batch=8):
  QT/KT = (Wq|bq)^T @ (q^T|1)  -> [256, 2048] fp16 (Q prescaled by 1/sqrt(256))
  For each 512-wide q-block (software-pipelined: scores of block k+1 are
  emitted before the AV pass of block k so ACT exp overlaps PE):
    S^T[k,q] tiles = KT^T-slice @ QT   (fp16 matmuls, f32 psum)
    E = exp(S^T) (ACT; no max subtraction: scores ~ N(0,1)) -> fp16
    E *= mask01 (DVE fp16 SBUF)
    out_aug[q, 0:258] = sum_k E^T-slice @ (V|bv|ones-col); col 256 = denom
  LayerNorm over h with eps*denom^2 trick (scale invariance) + gamma/beta
"""
import sys

sys.path.insert(0, "/opt/trn_rl_repo")

import numpy as np

import concourse.bacc as bacc
import concourse.tile as tile
from concourse import mybir
from concourse.bass_utils import run_bass_kernel_spmd

# Force a single ACT table set (covers Exp/Ln/Copy) so the table-load pass
# never thrashes between exp_and_others and natural_log_exp_and_others.
_orig_get_tables = bacc.get_activation_tables
def _single_set_tables(arch):
    tabs = _orig_get_tables(arch)
    return {name: (fns if name == "natural_log_exp_and_others" else set())
            for name, fns in tabs.items()}
bacc.get_activation_tables = _single_set_tables

F32 = mybir.dt.float32
F16 = mybir.dt.float16
AF = mybir.ActivationFunctionType
OP = mybir.AluOpType

S = 2048          # sequence length per batch
F = 96            # input feature dim
H = 256           # hidden dim
NCORES = 8
EPS = 1e-6
QB = 512          # q-block width
NBLK = S // QB    # 4
KT_TILES = 16     # 2048 / 128
QT_PER_BLK = QB // 128  # 4
FIN_BATCH = 2     # q-tiles finalized per batch
HA = H + 2        # augmented hidden: 256 vals + denom col + pad
FP = F + 2        # augmented feature rows: 96 + ones row + zero pad


def build_nc(identity_gb=False):
    nc = bacc.Bacc()

    qT_d = nc.dram_tensor("qT", [FP, S], F16, kind="ExternalInput")
    kT_d = nc.dram_tensor("kT", [FP, S], F16, kind="ExternalInput")
    vT_d = nc.dram_tensor("vT", [FP, S], F16, kind="ExternalInput")
    # mask packed per (q-block, ktgroup): [NBLK, 4, 128, 4, QB] so one DMA
    # covers 4 k-tiles with 4KB-contiguous per-partition runs
    mask_d = nc.dram_tensor("maskT", [NBLK, 4, 128, 4, QB], F16, kind="ExternalInput")
    wq_d = nc.dram_tensor("wq", [FP, H], F16, kind="ExternalInput")
    wk_d = nc.dram_tensor("wk", [FP, H], F16, kind="ExternalInput")
    wv_d = nc.dram_tensor("wv", [FP, HA], F16, kind="ExternalInput")
    gamma_d = nc.dram_tensor("gamma", [H], F32, kind="ExternalInput")
    beta_d = nc.dram_tensor("beta", [H], F32, kind="ExternalInput")
    out_d = nc.dram_tensor("out", [S, H], F32, kind="ExternalOutput")

    with tile.TileContext(nc) as tc:
        with (
            tc.tile_pool(name="consts", bufs=1) as consts,
            tc.tile_pool(name="mask", bufs=4) as maskp,
            tc.tile_pool(name="expT", bufs=34) as expp,
            tc.tile_pool(name="fin", bufs=3) as finp,
            tc.tile_pool(name="outp", bufs=3) as outp,
            tc.tile_pool(name="ocp", bufs=9) as ocpp,
            tc.tile_pool(name="ps_s", bufs=4, space="PSUM") as ps_s,
            tc.tile_pool(name="ps_a", bufs=4, space="PSUM") as ps_a,
        ):
            # ---- load params via HWDGE, 2 chunks each, Q first ----
            qT = consts.tile([FP, S], F16, name="qT", tag="qT")
            kTt = consts.tile([FP, S], F16, name="kTt", tag="kTt")
            vT = consts.tile([FP, S], F16, name="vT", tag="vT")
            wq = consts.tile([FP, H], F16, name="wq", tag="wq")
            wk = consts.tile([FP, H], F16, name="wk", tag="wk")
            wv = consts.tile([FP, HA], F16, name="wv", tag="wv")
            nc.sync.dma_start(out=wq, in_=wq_d[:, :])
            nc.sync.dma_start(out=qT[:, 0:1024], in_=qT_d[:, 0:1024])
            nc.sync.dma_start(out=qT[:, 1024:2048], in_=qT_d[:, 1024:2048])
            nc.sync.dma_start(out=wk, in_=wk_d[:, :])
            nc.sync.dma_start(out=kTt[:, 0:1024], in_=kT_d[:, 0:1024])
            nc.sync.dma_start(out=kTt[:, 1024:2048], in_=kT_d[:, 1024:2048])
            nc.sync.dma_start(out=wv, in_=wv_d[:, :])
            nc.sync.dma_start(out=vT[:, 0:1024], in_=vT_d[:, 0:1024])
            nc.sync.dma_start(out=vT[:, 1024:2048], in_=vT_d[:, 1024:2048])
            if not identity_gb:
                gam = consts.tile([128, H], F32, name="gam", tag="gam")
                bet = consts.tile([128, H], F32, name="bet", tag="bet")
                nc.sync.dma_start(out=gam, in_=gamma_d[:].partition_broadcast(128))
                nc.sync.dma_start(out=bet, in_=beta_d[:].partition_broadcast(128))

            # ---- projections: QT/KT [2][128, S] (h-major), V_aug [128, 16, 258] ----
            QT = [consts.tile([128, S], F16, name=f"QT{i}", tag=f"QT{i}") for i in range(2)]
            KT = [consts.tile([128, S], F16, name=f"KT{i}", tag=f"KT{i}") for i in range(2)]
            V_aug = consts.tile([128, KT_TILES, HA], F16, name="V_aug", tag="V_aug")

            for hc in range(2):
                for sc in range(4):
                    psq = ps_s.tile([128, 512], F32, name="psS", tag="psS")
                    nc.tensor.matmul(
                        out=psq,
                        lhsT=wq[:, hc * 128:(hc + 1) * 128],
                        rhs=qT[:, sc * 512:(sc + 1) * 512],
                        start=True, stop=True,
                    )
                    nc.scalar.activation(
                        out=QT[hc][:, sc * 512:(sc + 1) * 512], in_=psq, func=AF.Copy)
                    psk = ps_s.tile([128, 512], F32, name="psS", tag="psS")
                    nc.tensor.matmul(
                        out=psk,
                        lhsT=wk[:, hc * 128:(hc + 1) * 128],
                        rhs=kTt[:, sc * 512:(sc + 1) * 512],
                        start=True, stop=True,
                    )
                    nc.vector.tensor_copy(
                        out=KT[hc][:, sc * 512:(sc + 1) * 512], in_=psk)

            def emit_pass_a(blk, ets):
                mks = {}
                for kt in range(KT_TILES):
                    ktg, t = divmod(kt, 4)
                    if t == 0:
                        mk = maskp.tile([128, 4, QB], F16, name="mk", tag="mk")
                        nc.sync.dma_start(out=mk, in_=mask_d[blk, ktg])
                        scr = finp.tile([1, 1], F16, name="scr", tag="scr")
                        nc.vector.tensor_copy(out=scr, in_=mk[0:1, 0, 0:1])
                        mks[ktg] = mk
                    psS = ps_s.tile([128, QB], F32, name="psS", tag="psS")
                    for hc in range(2):
                        nc.tensor.matmul(
                            out=psS,
                            lhsT=KT[hc][:, kt * 128:(kt + 1) * 128],
                            rhs=QT[hc][:, blk * QB:(blk + 1) * QB],
                            start=(hc == 0), stop=(hc == 1),
                        )
                    et = expp.tile([128, QB], F16, name="et", tag="et")
                    nc.scalar.activation(out=et, in_=psS, func=AF.Exp)
                    eng = nc.vector if kt % 2 == 0 else nc.gpsimd
                    eng.tensor_tensor(out=et, in0=et, in1=mks[ktg][:, t, :], op=OP.mult)
                    ets.append(et)

            def emit_v_proj():
                for st in range(KT_TILES):
                    psv = ps_a.tile([128, HA], F32, name="acc", tag="acc")
                    nc.tensor.matmul(
                        out=psv,
                        lhsT=vT[:, st * 128:(st + 1) * 128],
                        rhs=wv,
                        start=True, stop=True,
                    )
                    nc.scalar.activation(out=V_aug[:, st, :], in_=psv, func=AF.Copy)

            def emit_pass_b(blk, ets, ocps):
                for qt in range(QT_PER_BLK):
                    acc = ps_a.tile([128, HA], F32, name="acc", tag="acc")
                    for kt in range(KT_TILES):
                        nc.tensor.matmul(
                            out=acc,
                            lhsT=ets[kt][:, qt * 128:(qt + 1) * 128],
                            rhs=V_aug[:, kt, :],
                            start=(kt == 0), stop=(kt == KT_TILES - 1),
                        )
                    # free the psum slot right away; finalize runs from SBUF
                    o_cp = ocpp.tile([128, HA], F32, name="o_cp", tag="o_cp")
                    nc.vector.tensor_copy(out=o_cp, in_=acc)
                    ocps.append(o_cp)

            def emit_finalize(blk, ocps):
                mv = finp.tile([128, QT_PER_BLK, 2], F32, name="mv", tag="mv")
                epsd = finp.tile([128, QT_PER_BLK], F32, name="epsd", tag="epsd")
                for j, o_cp in enumerate(ocps):
                    st6 = finp.tile([128, 6], F32, name="st6", tag="st6")
                    nc.vector.bn_stats(out=st6, in_=o_cp[:, 0:H])
                    nc.vector.bn_aggr(out=mv[:, j, :], in_=st6)
                    nc.vector.tensor_scalar(
                        out=epsd[:, j:j + 1], in0=o_cp[:, H:H + 1],
                        scalar1=o_cp[:, H:H + 1], scalar2=float(EPS),
                        op0=OP.mult, op1=OP.mult)
                ve = finp.tile([128, QT_PER_BLK], F32, name="ve", tag="ve")
                nc.vector.tensor_tensor(out=ve, in0=epsd, in1=mv[:, :, 1], op=OP.add)
                rstd = finp.tile([128, QT_PER_BLK], F32, name="rstd", tag="rstd")
                nc.scalar.activation(out=rstd, in_=ve, func=AF.Ln)
                nc.scalar.activation(out=rstd, in_=rstd, func=AF.Exp, scale=-0.5)
                for j, o_cp in enumerate(ocps):
                    o_n = outp.tile([128, H], F32, name="o_n", tag="o_n")
                    nc.vector.tensor_scalar(
                        out=o_n, in0=o_cp[:, 0:H],
                        scalar1=mv[:, j, 0:1], scalar2=rstd[:, j:j + 1],
                        op0=OP.subtract, op1=OP.mult)
                    if not identity_gb:
                        nc.gpsimd.tensor_tensor(out=o_n, in0=o_n, in1=gam, op=OP.mult)
                        nc.gpsimd.tensor_tensor(out=o_n, in0=o_n, in1=bet, op=OP.add)
                    row0 = blk * QB + j * 128
                    nc.sync.dma_start(out=out_d[row0:row0 + 128, :], in_=o_n)

            # ---- software-pipelined main loop (3 stages) ----
            ets_by_blk = {0: []}
            ocps_by_blk = {}
            emit_pass_a(0, ets_by_blk[0])
            emit_v_proj()
            for blk in range(1, NBLK + 2):
                if blk < NBLK:
                    ets_by_blk[blk] = []
                    emit_pass_a(blk, ets_by_blk[blk])
                elif blk - 2 >= 0:
                    # last stages have no pass_a: finalize first so its DVE/ACT
                    # work overlaps the remaining AV matmuls instead of sitting
                    # behind o_cp copies in the queue
                    emit_finalize(blk - 2, ocps_by_blk.pop(blk - 2))
                if blk - 1 < NBLK:
                    ocps_by_blk[blk - 1] = []
                    emit_pass_b(blk - 1, ets_by_blk.pop(blk - 1), ocps_by_blk[blk - 1])
                if blk < NBLK and blk - 2 >= 0:
                    emit_finalize(blk - 2, ocps_by_blk.pop(blk - 2))

    nc.finalize()
    return nc


_NC = {}


def _get_nc(identity_gb=False):
    if identity_gb not in _NC:
        _NC[identity_gb] = build_nc(identity_gb)
    return _NC[identity_gb]


def make_in_maps(query, key, value, mask, Wq, bq, Wk, bk, Wv, bv, gamma, beta):
    B = query.shape[0]
    scale = np.float32(1.0 / 16.0)  # 1/sqrt(H)
    zrow_h = np.zeros((1, H), dtype=np.float32)
    wq_aug = np.concatenate([Wq * scale, bq[None, :] * scale, zrow_h], 0).astype(np.float16)
    wk_aug = np.concatenate([Wk, bk[None, :], zrow_h], 0).astype(np.float16)
    wv_aug = np.zeros((FP, HA), dtype=np.float32)
    wv_aug[:F, :H] = Wv
    wv_aug[F, :H] = bv
    wv_aug[F, H] = 1.0
    wv_aug = wv_aug.astype(np.float16)
    gamma = np.ascontiguousarray(gamma.astype(np.float32))
    beta = np.ascontiguousarray(beta.astype(np.float32))

    ones_row = np.ones((1, S), dtype=np.float32)
    zero_row = np.zeros((1, S), dtype=np.float32)
    in_maps = []
    for b in range(B):
        qT = np.concatenate([query[b].T, ones_row, zero_row], 0).astype(np.float16)
        kT = np.concatenate([key[b].T, ones_row, zero_row], 0).astype(np.float16)
        vT = np.concatenate([value[b].T, ones_row, zero_row], 0).astype(np.float16)
        m01 = (mask[b].T != 0).astype(np.float16)           # [k, q]
        # k = ktg*512 + t*128 + p ; q = blk*QB + qq
        m01 = m01.reshape(4, 4, 128, NBLK, QB).transpose(3, 0, 2, 1, 4)
        m01 = np.ascontiguousarray(m01)                      # [blk, ktg, p, t, q]
        in_maps.append({
            "qT": np.ascontiguousarray(qT),
            "kT": np.ascontiguousarray(kT),
            "vT": np.ascontiguousarray(vT),
            "maskT": m01,
            "wq": wq_aug, "wk": wk_aug, "wv": wv_aug,
            "gamma": gamma, "beta": beta,
        })
    return in_maps


def kernel(query, key, value, mask, Wq, bq, Wk, bk, Wv, bv, gamma, beta):
    in_maps = make_in_maps(query, key, value, mask, Wq, bq, Wk, bk, Wv, bv,
                           gamma, beta)
    idgb = bool(np.all(gamma == 1.0) and np.all(beta == 0.0))
    nc = _get_nc(idgb)
    res = run_bass_kernel_spmd(nc, in_maps, list(range(NCORES)))
    out = np.stack([res.results[c]["out"] for c in range(NCORES)], axis=0)
    return out.astype(np.float32)


# revision 22
# speedup vs baseline: 1.0141x; 1.0141x over previous
"""Trainium2 Bass kernel for masked attention + LayerNorm (nn_Attention_4183298146361).

Per-core (data-parallel over batch=8):
  QT/KT = (Wq|bq)^T @ (q^T|1)  -> [256, 2048] fp16 (Q prescaled by 1/sqrt(256))
  For each 512-wide q-block (software-pipelined: scores of block k+1 are
  emitted before the AV pass of block k so ACT exp overlaps PE):
    S^T[k,q] tiles = KT^T-slice @ QT   (fp16 matmuls, f32 psum)
    E = exp(S^T) (ACT; no max subtraction: scores ~ N(0,1)) -> fp16
    E *= mask01 (DVE fp16 SBUF)
    out_aug[q, 0:258] = sum_k E^T-slice @ (V|bv|ones-col); col 256 = denom
  LayerNorm over h with eps*denom^2 trick (scale invariance) + gamma/beta
"""
import sys

sys.path.insert(0, "/opt/trn_rl_repo")

import numpy as np

import concourse.bacc as bacc
import concourse.tile as tile
from concourse import mybir
from concourse.bass_utils import run_bass_kernel_spmd

# Force a single ACT table set (covers Exp/Ln/Copy) so the table-load pass
# never thrashes between exp_and_others and natural_log_exp_and_others.
_orig_get_tables = bacc.get_activation_tables
def _single_set_tables(arch):
    tabs = _orig_get_tables(arch)
    return {name: (fns if name == "natural_log_exp_and_others" else set())
            for name, fns in tabs.items()}
bacc.get_activation_tables = _single_set_tables

F32 = mybir.dt.float32
F16 = mybir.dt.float16
AF = mybir.ActivationFunctionType
OP = mybir.AluOpType

S = 2048          # sequence length per batch
F = 96            # input feature dim
H = 256           # hidden dim
NCORES = 8
EPS = 1e-6
QB = 512          # q-block width
NBLK = S // QB    # 4
KT_TILES = 16     # 2048 / 128
QT_PER_BLK = QB // 128  # 4
FIN_BATCH = 2     # q-tiles finalized per batch
HA = H + 2        # augmented hidden: 256 vals + denom col + pad
FP = F + 2        # augmented feature rows: 96 + ones row + zero pad


def build_nc(identity_gb=False):
    nc = bacc.Bacc()

    qT_d = nc.dram_tensor("qT", [FP, S], F16, kind="ExternalInput")
    kT_d = nc.dram_tensor("kT", [FP, S], F16, kind="ExternalInput")
    vT_d = nc.dram_tensor("vT", [FP, S], F16, kind="ExternalInput")
    # mask packed per (q-block, ktgroup): [NBLK, 4, 128, 4, QB] so one DMA
    # covers 4 k-tiles with 4KB-contiguous per-partition runs
    mask_d = nc.dram_tensor("maskT", [NBLK, 4, 128, 4, QB], F16, kind="ExternalInput")
    wq_d = nc.dram_tensor("wq", [FP, H], F16, kind="ExternalInput")
    wk_d = nc.dram_tensor("wk", [FP, H], F16, kind="ExternalInput")
    wv_d = nc.dram_tensor("wv", [FP, HA], F16, kind="ExternalInput")
    gamma_d = nc.dram_tensor("gamma", [H], F32, kind="ExternalInput")
    beta_d = nc.dram_tensor("beta", [H], F32, kind="ExternalInput")
    out_d = nc.dram_tensor("out", [S, H], F32, kind="ExternalOutput")

    with tile.TileContext(nc) as tc:
        with (
            tc.tile_pool(name="consts", bufs=1) as consts,
            tc.tile_pool(name="mask", bufs=4) as maskp,
            tc.tile_pool(name="expT", bufs=34) as expp,
            tc.tile_pool(name="fin", bufs=3) as finp,
            tc.tile_pool(name="outp", bufs=3) as outp,
            tc.tile_pool(name="ocp", bufs=9) as ocpp,
            tc.tile_pool(name="ps_s", bufs=4, space="PSUM") as ps_s,
            tc.tile_pool(name="ps_a", bufs=4, space="PSUM") as ps_a,
        ):
            # ---- load params via HWDGE, 2 chunks each, Q first ----
            qT = consts.tile([FP, S], F16, name="qT", tag="qT")
            kTt = consts.tile([FP, S], F16, name="kTt", tag="kTt")
            vT = consts.tile([FP, S], F16, name="vT", tag="vT")
            wq = consts.tile([FP, H], F16, name="wq", tag="wq")
            wk = consts.tile([FP, H], F16, name="wk", tag="wk")
            wv = consts.tile([FP, HA], F16, name="wv", tag="wv")
            nc.sync.dma_start(out=wq, in_=wq_d[:, :])
            nc.sync.dma_start(out=qT[:, 0:1024], in_=qT_d[:, 0:1024])
            nc.sync.dma_start(out=qT[:, 1024:2048], in_=qT_d[:, 1024:2048])
            nc.sync.dma_start(out=wk, in_=wk_d[:, :])
            nc.sync.dma_start(out=kTt[:, 0:1024], in_=kT_d[:, 0:1024])
            nc.sync.dma_start(out=kTt[:, 1024:2048], in_=kT_d[:, 1024:2048])
            nc.sync.dma_start(out=wv, in_=wv_d[:, :])
            nc.sync.dma_start(out=vT[:, 0:1024], in_=vT_d[:, 0:1024])
            nc.sync.dma_start(out=vT[:, 1024:2048], in_=vT_d[:, 1024:2048])
            if not identity_gb:
                gam = consts.tile([128, H], F32, name="gam", tag="gam")
                bet = consts.tile([128, H], F32, name="bet", tag="bet")
                nc.sync.dma_start(out=gam, in_=gamma_d[:].partition_broadcast(128))
                nc.sync.dma_start(out=bet, in_=beta_d[:].partition_broadcast(128))

            # ---- projections: QT/KT [2][128, S] (h-major), V_aug [128, 16, 258] ----
            QT = [consts.tile([128, S], F16, name=f"QT{i}", tag=f"QT{i}") for i in range(2)]
            KT = [consts.tile([128, S], F16, name=f"KT{i}", tag=f"KT{i}") for i in range(2)]
            V_aug = consts.tile([128, KT_TILES, HA], F16, name="V_aug", tag="V_aug")

            for hc in range(2):
                for sc in range(4):
                    psq = ps_s.tile([128, 512], F32, name="psS", tag="psS")
                    nc.tensor.matmul(
                        out=psq,
                        lhsT=wq[:, hc * 128:(hc + 1) * 128],
                        rhs=qT[:, sc * 512:(sc + 1) * 512],
                        start=True, stop=True,
                    )
                    nc.scalar.activation(
                        out=QT[hc][:, sc * 512:(sc + 1) * 512], in_=psq, func=AF.Copy)
                    psk = ps_s.tile([128, 512], F32, name="psS", tag="psS")
                    nc.tensor.matmul(
                        out=psk,
                        lhsT=wk[:, hc * 128:(hc + 1) * 128],
                        rhs=kTt[:, sc * 512:(sc + 1) * 512],
                        start=True, stop=True,
                    )
                    nc.vector.tensor_copy(
                        out=KT[hc][:, sc * 512:(sc + 1) * 512], in_=psk)

            def emit_pass_a(blk, ets):
                mks = {}
                for kt in range(KT_TILES):
                    ktg, t = divmod(kt, 4)
                    if t == 0:
                        mk = maskp.tile([128, 4, QB], F16, name="mk", tag="mk")
                        nc.sync.dma_start(out=mk, in_=mask_d[blk, ktg])
                        scr = finp.tile([1, 1], F16, name="scr", tag="scr")
                        nc.vector.tensor_copy(out=scr, in_=mk[0:1, 0, 0:1])
                        mks[ktg] = mk
                    psS = ps_s.tile([128, QB], F32, name="psS", tag="psS")
                    for hc in range(2):
                        nc.tensor.matmul(
                            out=psS,
                            lhsT=KT[hc][:, kt * 128:(kt + 1) * 128],
                            rhs=QT[hc][:, blk * QB:(blk + 1) * QB],
                            start=(hc == 0), stop=(hc == 1),
                        )
                    et = expp.tile([128, QB], F16, name="et", tag="et")
                    nc.scalar.activation(out=et, in_=psS, func=AF.Exp)
                    eng = nc.vector if kt % 2 == 0 else nc.gpsimd
                    eng.tensor_tensor(out=et, in0=et, in1=mks[ktg][:, t, :], op=OP.mult)
                    ets.append(et)

            def emit_v_proj():
                for st in range(KT_TILES):
                    psv = ps_a.tile([128, HA], F32, name="acc", tag="acc")
                    nc.tensor.matmul(
                        out=psv,
                        lhsT=vT[:, st * 128:(st + 1) * 128],
                        rhs=wv,
                        start=True, stop=True,
                    )
                    nc.scalar.activation(out=V_aug[:, st, :], in_=psv, func=AF.Copy)

            def emit_pass_b(blk, ets, ocps):
                for qt in range(QT_PER_BLK):
                    acc = ps_a.tile([128, HA], F32, name="acc", tag="acc")
                    for kt in range(KT_TILES):
                        nc.tensor.matmul(
                            out=acc,
                            lhsT=ets[kt][:, qt * 128:(qt + 1) * 128],
                            rhs=V_aug[:, kt, :],
                            start=(kt == 0), stop=(kt == KT_TILES - 1),
                        )
                    # free the psum slot right away; finalize runs from SBUF
                    o_cp = ocpp.tile([128, HA], F32, name="o_cp", tag="o_cp")
                    nc.vector.tensor_copy(out=o_cp, in_=acc)
                    ocps.append(o_cp)

            def emit_finalize(blk, ocps):
                mv = finp.tile([128, QT_PER_BLK, 2], F32, name="mv", tag="mv")
                epsd = finp.tile([128, QT_PER_BLK], F32, name="epsd", tag="epsd")
                for j, o_cp in enumerate(ocps):
                    st6 = finp.tile([128, 6], F32, name="st6", tag="st6")
                    nc.vector.bn_stats(out=st6, in_=o_cp[:, 0:H])
                    nc.vector.bn_aggr(out=mv[:, j, :], in_=st6)
                    nc.vector.tensor_scalar(
                        out=epsd[:, j:j + 1], in0=o_cp[:, H:H + 1],
                        scalar1=o_cp[:, H:H + 1], scalar2=float(EPS),
                        op0=OP.mult, op1=OP.mult)
                ve = finp.tile([128, QT_PER_BLK], F32, name="ve", tag="ve")
                nc.vector.tensor_tensor(out=ve, in0=epsd, in1=mv[:, :, 1], op=OP.add)
                rstd = finp.tile([128, QT_PER_BLK], F32, name="rstd", tag="rstd")
                nc.scalar.activation(out=rstd, in_=ve, func=AF.Ln)
                nc.scalar.activation(out=rstd, in_=rstd, func=AF.Exp, scale=-0.5)
                for j, o_cp in enumerate(ocps):
                    o_n = outp.tile([128, H], F32, name="o_n", tag="o_n")
                    nc.vector.tensor_scalar(
                        out=o_n, in0=o_cp[:, 0:H],
                        scalar1=mv[:, j, 0:1], scalar2=rstd[:, j:j + 1],
                        op0=OP.subtract, op1=OP.mult)
                    if not identity_gb:
                        nc.gpsimd.tensor_tensor(out=o_n, in0=o_n, in1=gam, op=OP.mult)
                        nc.gpsimd.tensor_tensor(out=o_n, in0=o_n, in1=bet, op=OP.add)
                    row0 = blk * QB + j * 128
                    nc.sync.dma_start(out=out_d[row0:row0 + 128, :], in_=o_n)

            def emit_finalize_perqt(blk, ocps):
                for j, o_cp in enumerate(ocps):
                    st6 = finp.tile([128, 6], F32, name="st6", tag="st6")
                    nc.vector.bn_stats(out=st6, in_=o_cp[:, 0:H])
                    mv1 = finp.tile([128, 2], F32, name="mv1", tag="mv1")
                    nc.vector.bn_aggr(out=mv1, in_=st6)
                    ve1 = finp.tile([128, 1], F32, name="ve1", tag="ve1")
                    nc.vector.tensor_scalar(
                        out=ve1, in0=o_cp[:, H:H + 1],
                        scalar1=o_cp[:, H:H + 1], scalar2=float(EPS),
                        op0=OP.mult, op1=OP.mult)
                    nc.vector.tensor_tensor(out=ve1, in0=ve1, in1=mv1[:, 1:2], op=OP.add)
                    rstd1 = finp.tile([128, 1], F32, name="rstd1", tag="rstd1")
                    nc.scalar.activation(out=rstd1, in_=ve1, func=AF.Ln)
                    nc.scalar.activation(out=rstd1, in_=rstd1, func=AF.Exp, scale=-0.5)
                    o_n = outp.tile([128, H], F32, name="o_n", tag="o_n")
                    nc.vector.tensor_scalar(
                        out=o_n, in0=o_cp[:, 0:H],
                        scalar1=mv1[:, 0:1], scalar2=rstd1,
                        op0=OP.subtract, op1=OP.mult)
                    if not identity_gb:
                        nc.gpsimd.tensor_tensor(out=o_n, in0=o_n, in1=gam, op=OP.mult)
                        nc.gpsimd.tensor_tensor(out=o_n, in0=o_n, in1=bet, op=OP.add)
                    row0 = blk * QB + j * 128
                    nc.sync.dma_start(out=out_d[row0:row0 + 128, :], in_=o_n)

            # ---- software-pipelined main loop (3 stages) ----
            ets_by_blk = {0: []}
            ocps_by_blk = {}
            emit_pass_a(0, ets_by_blk[0])
            emit_v_proj()
            for blk in range(1, NBLK + 2):
                if blk < NBLK:
                    ets_by_blk[blk] = []
                    emit_pass_a(blk, ets_by_blk[blk])
                elif blk - 2 >= 0:
                    # last stages have no pass_a: finalize first so its DVE/ACT
                    # work overlaps the remaining AV matmuls instead of sitting
                    # behind o_cp copies in the queue
                    if blk - 2 == NBLK - 1:
                        emit_finalize_perqt(blk - 2, ocps_by_blk.pop(blk - 2))
                    else:
                        emit_finalize(blk - 2, ocps_by_blk.pop(blk - 2))
                if blk - 1 < NBLK:
                    ocps_by_blk[blk - 1] = []
                    emit_pass_b(blk - 1, ets_by_blk.pop(blk - 1), ocps_by_blk[blk - 1])
                if blk < NBLK and blk - 2 >= 0:
                    emit_finalize(blk - 2, ocps_by_blk.pop(blk - 2))

    nc.finalize()
    return nc


_NC = {}


def _get_nc(identity_gb=False):
    if identity_gb not in _NC:
        _NC[identity_gb] = build_nc(identity_gb)
    return _NC[identity_gb]


def make_in_maps(query, key, value, mask, Wq, bq, Wk, bk, Wv, bv, gamma, beta):
    B = query.shape[0]
    scale = np.float32(1.0 / 16.0)  # 1/sqrt(H)
    zrow_h = np.zeros((1, H), dtype=np.float32)
    wq_aug = np.concatenate([Wq * scale, bq[None, :] * scale, zrow_h], 0).astype(np.float16)
    wk_aug = np.concatenate([Wk, bk[None, :], zrow_h], 0).astype(np.float16)
    wv_aug = np.zeros((FP, HA), dtype=np.float32)
    wv_aug[:F, :H] = Wv
    wv_aug[F, :H] = bv
    wv_aug[F, H] = 1.0
    wv_aug = wv_aug.astype(np.float16)
    gamma = np.ascontiguousarray(gamma.astype(np.float32))
    beta = np.ascontiguousarray(beta.astype(np.float32))

    ones_row = np.ones((1, S), dtype=np.float32)
    zero_row = np.zeros((1, S), dtype=np.float32)
    in_maps = []
    for b in range(B):
        qT = np.concatenate([query[b].T, ones_row, zero_row], 0).astype(np.float16)
        kT = np.concatenate([key[b].T, ones_row, zero_row], 0).astype(np.float16)
        vT = np.concatenate([value[b].T, ones_row, zero_row], 0).astype(np.float16)
        m01 = (mask[b].T != 0).astype(np.float16)           # [k, q]
        # k = ktg*512 + t*128 + p ; q = blk*QB + qq
        m01 = m01.reshape(4, 4, 128, NBLK, QB).transpose(3, 0, 2, 1, 4)
        m01 = np.ascontiguousarray(m01)                      # [blk, ktg, p, t, q]
        in_maps.append({
            "qT": np.ascontiguousarray(qT),
            "kT": np.ascontiguousarray(kT),
            "vT": np.ascontiguousarray(vT),
            "maskT": m01,
            "wq": wq_aug, "wk": wk_aug, "wv": wv_aug,
            "gamma": gamma, "beta": beta,
        })
    return in_maps


def kernel(query, key, value, mask, Wq, bq, Wk, bk, Wv, bv, gamma, beta):
    in_maps = make_in_maps(query, key, value, mask, Wq, bq, Wk, bk, Wv, bv,
                           gamma, beta)
    idgb = bool(np.all(gamma == 1.0) and np.all(beta == 0.0))
    nc = _get_nc(idgb)
    res = run_bass_kernel_spmd(nc, in_maps, list(range(NCORES)))
    out = np.stack([res.results[c]["out"] for c in range(NCORES)], axis=0)
    return out.astype(np.float32)
